# revision 1
# baseline (speedup 1.0000x reference)
"""Trainium2 Bass kernel for a transformer MiniBlock (B=4, T=2048, C=1024, 16 heads,
causal attention, 4x FFN), sharded over 8 NeuronCores.

Sharding: core = (batch b=core//2, role r=core%2). Each core runs the full block for
1024 of its batch's 2048 tokens (four 256-token chunks, chosen to balance causal
attention work), computing k/v over the full sequence (no cross-core communication).
The program is SPMD-uniform: loop bounds use the max k-window per chunk slot, and
per-core causal masks (input data) zero out the difference.

All matmuls run as float32r (full PE rate at moving free-dim >= 256, near-fp32
precision). Activations stay channel-major ("transposed") end to end, so layernorm
statistics, softmax sums and broadcasts are all done with small matmuls
(ones-vector / row-select tricks); the attention softmax is computed k-major (S^T)
with a ones-column appended to V so the softmax denominators fall out of the same
matmul that computes attn@V, and every bias is a per-partition column folded into
the PSUM-eviction op.
"""
import sys

sys.path.insert(0, "/opt/trn_rl_repo")

import numpy as np
from contextlib import ExitStack

import concourse.bacc as bacc
import concourse.mybir as mybir
import concourse.tile as tile

F32 = mybir.dt.float32
F32R = mybir.dt.float32r
AF = mybir.ActivationFunctionType
ALU = mybir.AluOpType

P = 128
T = 2048          # full sequence
C = 1024          # embedding
NQ = 1024         # query tokens per core
H4 = 4096         # ffn hidden
NH = 16
HS = 64
NPAIR = 8         # head pairs
KC = C // P       # 8 channel tiles
NSLOT = 4         # 256-token query chunks per core
CW = 256          # chunk width
NKT_PROG = [4, 16, 8, 12]          # k-tiles per slot (program constant, max over roles)
GSETS = [[0, 7, 2, 5], [1, 6, 3, 4]]  # global 256-chunk index per slot, per role
LN_EPS = 1e-5
SCALE = float(HS) ** -0.5  # 0.125


def r32(ap):
    return ap.bitcast(F32R)


def _build():
    nc = bacc.Bacc(None, target_bir_lowering=False, debug=False)
    names = {}
    with tile.TileContext(nc) as tc, ExitStack() as top:
        dram = top.enter_context(tc.tile_pool(name="io", bufs=1, space="DRAM"))

        def din(name, shape, dt=F32):
            t = dram.tile(shape, dt, kind="ExternalInput", name=f"i_{name}")
            names[name] = t.name
            return t

        xT_d = din("xT", [C, T], F32R)
        xTq_d = din("xTq", [C, NQ], F32R)
        Wq_d = din("Wq", [C, C], F32R)
        Wk_d = din("Wk", [C, C], F32R)
        Wv_d = din("Wv", [C, C], F32R)
        Wo_d = din("Wo", [C, C], F32R)
        W1_d = din("W1", [C, H4], F32R)
        W2_d = din("W2", [H4, C], F32R)
        bot_d = din("bot", [P, KC])      # bo as per-partition cols, col m
        b1t_d = din("b1t", [P, 32])      # b1 cols, col mm (hidden tile index)
        b2t_d = din("b2t", [P, KC])
        g1_d = din("g1c", [P, KC])
        be1_d = din("be1c", [P, KC])
        g2_d = din("g2c", [P, KC])
        be2_d = din("be2c", [P, KC])
        onesC_d = din("onesC", [P, 1], F32R)
        onesP_d = din("onesP", [P, P], F32R)
        masks_d = din("masks", [NSLOT, 4, P, CW])

        out_d = dram.tile([C, NQ], F32, kind="ExternalOutput", name="o_out")
        names["out"] = out_d.name

        scr = top.enter_context(tc.tile_pool(name="scr", bufs=1, space="DRAM"))
        v65_dram = scr.tile([T // P, P, NPAIR, 2, 65], F32R, name="v65_dram")
        oT_dram = scr.tile([C, NQ], F32, name="oT_dram")

        # ---- psum pools (8 banks total: 2 + 2 + 4) ----
        pA = top.enter_context(tc.tile_pool(name="pA", bufs=2, space="PSUM"))
        pS = top.enter_context(tc.tile_pool(name="pS", bufs=2, space="PSUM"))
        pAV = top.enter_context(tc.tile_pool(name="pAV", bufs=1, space="PSUM"))

        # ---- persistent small sbuf ----
        pers = top.enter_context(tc.tile_pool(name="pers", bufs=1))
        onesC = pers.tile([P, 1], F32R, tag="onesC")
        nc.sync.dma_start(out=onesC[:], in_=onesC_d[:])
        onesP = pers.tile([P, P], F32R, tag="onesP")
        nc.sync.dma_start(out=onesP[:], in_=onesP_d[:])
        bot = pers.tile([P, KC], F32, tag="bot")
        nc.sync.dma_start(out=bot[:], in_=bot_d[:])
        b1t = pers.tile([P, 32], F32, tag="b1t")
        nc.sync.dma_start(out=b1t[:], in_=b1t_d[:])
        b2t = pers.tile([P, KC], F32, tag="b2t")
        nc.sync.dma_start(out=b2t[:], in_=b2t_d[:])
        lncol = {}
        for nm, d in [("g1", g1_d), ("be1", be1_d), ("g2", g2_d), ("be2", be2_d)]:
            t = pers.tile([P, KC], F32, tag=f"ln_{nm}", name=f"ln_{nm}")
            nc.sync.dma_start(out=t[:], in_=d[:])
            lncol[nm] = t
        zero_col = pers.tile([P, 1], F32, tag="zero_col")
        nc.vector.memset(zero_col[:], 0.0)
        eps_col = pers.tile([P, 1], F32, tag="eps_col")
        nc.vector.memset(eps_col[:], LN_EPS)

        def wstream(pool, wd, m, tag):
            """[P, KC, P] stationary tile: all channel k-tiles of W[:, m*P:(m+1)*P]."""
            w = pool.tile([P, KC, P], F32R, tag=tag, name=tag)
            nc.sync.dma_start(
                out=w[:], in_=wd[:, m * P:(m + 1) * P]
                .rearrange("(kc p) m -> p kc m", p=P))
            return w

        # =====================================================================
        # transposed-layout layernorm (in place unless out_tiles given)
        # =====================================================================
        def layernorm_T(xtiles, n, gname, bname, out_tag, out_tiles=None):
            ctx = ExitStack()
            work = ctx.enter_context(tc.tile_pool(name=f"lnw_{out_tag}", bufs=2))
            stat = ctx.enter_context(tc.tile_pool(name=f"lns_{out_tag}", bufs=1))
            nn = n // 512
            mu_row = stat.tile([1, n], F32R, tag="mu_row")
            msq_row = stat.tile([1, n], F32R, tag="msq_row")
            for i in range(nn):
                s = slice(i * 512, (i + 1) * 512)
                ps_sum = pAV.tile([1, 512], F32, tag="av0", name="ps_sum")
                ps_sq = pAV.tile([1, 512], F32, tag="av1", name="ps_sq")
                for kc in range(KC):
                    sq = work.tile([P, 512], F32R, tag="sq", name="sq")
                    nc.vector.tensor_tensor(sq[:], xtiles[kc][:, s], xtiles[kc][:, s],
                                            ALU.mult)
                    nc.tensor.matmul(ps_sum[:], r32(onesC[:]), r32(xtiles[kc][:, s]),
                                     start=(kc == 0), stop=(kc == KC - 1),
                                     skip_group_check=True)
                    nc.tensor.matmul(ps_sq[:], r32(onesC[:]), r32(sq[:]),
                                     start=(kc == 0), stop=(kc == KC - 1),
                                     skip_group_check=True)
                nc.scalar.activation(mu_row[:, s], ps_sum[:], AF.Copy, scale=1.0 / C)
                nc.scalar.activation(msq_row[:, s], ps_sq[:], AF.Copy, scale=1.0 / C)
            mu_b = stat.tile([P, n], F32, tag="mu_b")
            rstd_b = stat.tile([P, n], F32, tag="rstd_b")
            for i in range(nn):
                s = slice(i * 512, (i + 1) * 512)
                bps = pS.tile([P, 512], F32, tag="st", name="bps")
                nc.tensor.matmul(bps[:], r32(onesP[0:1, :]), r32(mu_row[:, s]),
                                 start=True, stop=True)
                nc.vector.tensor_copy(mu_b[:, s], bps[:])
                tmp = work.tile([P, 512], F32, tag="mutmp", name="mutmp")
                nc.vector.tensor_tensor(tmp[:], mu_b[:, s], mu_b[:, s], ALU.mult)
                bps2 = pS.tile([P, 512], F32, tag="st", name="bps2")
                nc.tensor.matmul(bps2[:], r32(onesP[0:1, :]), r32(msq_row[:, s]),
                                 start=True, stop=True)
                var = work.tile([P, 512], F32, tag="var", name="var")
                nc.vector.tensor_tensor(var[:], bps2[:], tmp[:], ALU.subtract)
                nc.scalar.activation(var[:], var[:], AF.Sqrt, bias=eps_col[:])
                nc.vector.reciprocal(rstd_b[:, s], var[:])
            outs = []
            for kc in range(KC):
                o = out_tiles[kc] if out_tiles is not None else xtiles[kc]
                nc.vector.tensor_tensor(o[:], xtiles[kc][:], mu_b[:], ALU.subtract)
                nc.vector.tensor_tensor(o[:], o[:], rstd_b[:], ALU.mult)
                nc.vector.tensor_scalar(o[:], o[:], lncol[gname][:, kc:kc + 1],
                                        lncol[bname][:, kc:kc + 1], ALU.mult, ALU.add)
                outs.append(o)
            ctx.close()
            return outs

        # =====================================================================
        # Phases 1-3 share a scope: qT/kT persist through attention
        # =====================================================================
        with ExitStack() as ph23:
            p_qT = ph23.enter_context(tc.tile_pool(name="p_qT", bufs=1))
            p_kT = ph23.enter_context(tc.tile_pool(name="p_kT", bufs=1))

            # q path first: LN1 of the core's own query columns, project, free
            qT = []
            with ExitStack() as phq:
                p_xTq = phq.enter_context(tc.tile_pool(name="p_xTq", bufs=1))
                xTq = []
                for kc in range(KC):
                    tq = p_xTq.tile([P, NQ], F32R, tag=f"xTq{kc}", name=f"xTq{kc}")
                    nc.sync.dma_start(out=tq[:], in_=xTq_d[kc * P:(kc + 1) * P, :])
                    xTq.append(tq)
                ln1q = layernorm_T(xTq, NQ, "g1", "be1", "ln1q")
                wqp = phq.enter_context(tc.tile_pool(name="wqp", bufs=2))
                for m in range(NPAIR):
                    wqm = wstream(wqp, Wq_d, m, "wqm")
                    qt = p_qT.tile([P, NQ], F32R, tag=f"qT{m}", name=f"qT{m}")
                    for nq in range(NQ // 512):
                        s = slice(nq * 512, (nq + 1) * 512)
                        ps = pA.tile([P, 512], F32, tag="proj", name="ps")
                        for kc in range(KC):
                            nc.tensor.matmul(ps[:], r32(wqm[:, kc, :]),
                                             r32(ln1q[kc][:, s]),
                                             start=(kc == 0), stop=(kc == KC - 1))
                        nc.vector.tensor_copy(qt[:, s], ps[:])
                    qT.append(qt)

            # k and v paths: LN1 over the full sequence
            with ExitStack() as phk:
                p_xT = phk.enter_context(tc.tile_pool(name="p_xT", bufs=1))
                xT = []
                for kc in range(KC):
                    t = p_xT.tile([P, T], F32R, tag=f"xT{kc}", name=f"xT{kc}")
                    nc.sync.dma_start(out=t[:], in_=xT_d[kc * P:(kc + 1) * P, :])
                    xT.append(t)
                ln1T = layernorm_T(xT, T, "g1", "be1", "ln1T")

                kT = []
                with ExitStack() as phkw:
                    wkp = phkw.enter_context(tc.tile_pool(name="wkp", bufs=2))
                    for m in range(NPAIR):
                        wkm = wstream(wkp, Wk_d, m, "wkm")
                        kt_t = p_kT.tile([P, T], F32R, tag=f"kT{m}", name=f"kT{m}")
                        for n in range(T // 512):
                            s = slice(n * 512, (n + 1) * 512)
                            ps = pA.tile([P, 512], F32, tag="proj", name="ps")
                            for kc in range(KC):
                                nc.tensor.matmul(ps[:], r32(wkm[:, kc, :]),
                                                 r32(ln1T[kc][:, s]),
                                                 start=(kc == 0), stop=(kc == KC - 1))
                            nc.vector.tensor_copy(kt_t[:, s], ps[:])
                        kT.append(kt_t)

                # v: token-major, 65-strided with ones col, spilled to dram
                with ExitStack() as phv:
                    wvp = phv.enter_context(tc.tile_pool(name="wvp", bufs=1))
                    vstage = phv.enter_context(tc.tile_pool(name="vstage", bufs=3))
                    for n in range(2):
                        wvn = wvp.tile([P, KC, 512], F32R, tag="wvn", name="wvn")
                        nc.sync.dma_start(
                            out=wvn[:], in_=Wv_d[:, n * 512:(n + 1) * 512]
                            .rearrange("(kc p) m -> p kc m", p=P))
                        for tt in range(T // P):
                            ps = pA.tile([P, 512], F32, tag="proj", name="ps")
                            for kc in range(KC):
                                nc.tensor.matmul(
                                    ps[:], r32(ln1T[kc][:, tt * P:(tt + 1) * P]),
                                    r32(wvn[:, kc, :]),
                                    start=(kc == 0), stop=(kc == KC - 1))
                            vst = vstage.tile([P, 4, 2, 65], F32R, tag="vst", name="vst")
                            nc.vector.tensor_copy(vst[:, :, :, 64:65], onesP[:, 0:8])
                            nc.vector.tensor_copy(
                                vst[:, :, :, 0:64],
                                ps[:].rearrange("p (pr par d) -> p pr par d",
                                                pr=4, par=2))
                            nc.sync.dma_start(
                                out=v65_dram[tt, :, 4 * n:4 * (n + 1)], in_=vst[:])

            # Phase 3: attention (xT freed; masks/streams fit above qT/kT)
            ph3 = ph23.enter_context(ExitStack())
            p_mask = ph3.enter_context(tc.tile_pool(name="p_mask", bufs=1))
            masks = {}
            for s in range(NSLOT):
                for j in range(4):
                    mt = p_mask.tile([P, CW], F32, tag=f"mask{s}_{j}",
                                     name=f"mask{s}_{j}")
                    nc.sync.dma_start(out=mt[:], in_=masks_d[s, j])
                    masks[(s, j)] = mt
            wop = ph3.enter_context(tc.tile_pool(name="wop", bufs=2))
            vs_pool = ph3.enter_context(tc.tile_pool(name="vs", bufs=3))
            pt_pool = ph3.enter_context(tc.tile_pool(name="pt", bufs=4))
            avn_pool = ph3.enter_context(tc.tile_pool(name="avn", bufs=2))
            sm_pool = ph3.enter_context(tc.tile_pool(name="sm", bufs=3))
            ost_pool = ph3.enter_context(tc.tile_pool(name="ost", bufs=3))

            for s in range(NSLOT):
                nkt = NKT_PROG[s]
                qs = slice(s * CW, (s + 1) * CW)
                avn = {}
                for g in range(4):
                    avps = [pAV.tile([65, CW], F32, tag=f"av{i}", name=f"avps{i}")
                            for i in range(4)]
                    for kt in range(nkt):
                        vt = vs_pool.tile([P, 2, 2, 65], F32R, tag="v", name="vt")
                        nc.sync.dma_start(out=vt[:],
                                          in_=v65_dram[kt, :, 2 * g:2 * (g + 1)])
                        for pp in range(2):
                            pair = 2 * g + pp
                            kws = slice(kt * P, (kt + 1) * P)
                            pse = pS.tile([P, CW], F32, tag="st", name="pse")
                            pso = pS.tile([P, CW], F32, tag="st", name="pso")
                            nc.tensor.matmul(pse[:], r32(kT[pair][0:64, kws]),
                                             r32(qT[pair][0:64, qs]),
                                             start=True, stop=True)
                            nc.tensor.matmul(pso[:], r32(kT[pair][64:128, kws]),
                                             r32(qT[pair][64:128, qs]),
                                             start=True, stop=True)
                            pte = pt_pool.tile([P, CW], F32R, tag="pt", name="pte")
                            pto = pt_pool.tile([P, CW], F32R, tag="pt", name="pto")
                            nc.scalar.activation(pte[:], pse[:], AF.Exp,
                                                 bias=zero_col[:], scale=SCALE)
                            nc.scalar.activation(pto[:], pso[:], AF.Exp,
                                                 bias=zero_col[:], scale=SCALE)
                            if kt >= nkt - 4:
                                mt = masks[(s, kt - (nkt - 4))]
                                nc.vector.tensor_tensor(pte[:], pte[:], mt[:],
                                                        ALU.mult)
                                nc.vector.tensor_tensor(pto[:], pto[:], mt[:],
                                                        ALU.mult)
                            st = (kt == 0)
                            sp = (kt == nkt - 1)
                            nc.tensor.matmul(avps[2 * pp][0:65, :],
                                             r32(vt[:, pp, 0, :]), r32(pte[:]),
                                             start=st, stop=sp, skip_group_check=True)
                            nc.tensor.matmul(avps[2 * pp + 1][0:65, :],
                                             r32(vt[:, pp, 1, :]), r32(pto[:]),
                                             start=st, stop=sp, skip_group_check=True)
                    for pp in range(2):
                        pair = 2 * g + pp
                        # evict both heads' AV + sums row to sbuf (in-lane)
                        ane = sm_pool.tile([65, CW], F32R, tag="ane", name="ane")
                        nc.vector.tensor_copy(ane[:], avps[2 * pp][0:65, :])
                        ano = sm_pool.tile([65, CW], F32R, tag="ano", name="ano")
                        nc.vector.tensor_copy(ano[:], avps[2 * pp + 1][0:65, :])
                        # broadcast sums row (partition 64 -> 64 rows) and invert
                        bce = pS.tile([64, CW], F32, tag="st", name="bce")
                        nc.tensor.matmul(bce[:], r32(onesP[64:65, 0:64]),
                                         r32(ane[64:65, :]), start=True, stop=True)
                        bco = pS.tile([64, CW], F32, tag="st", name="bco")
                        nc.tensor.matmul(bco[:], r32(onesP[64:65, 0:64]),
                                         r32(ano[64:65, :]), start=True, stop=True)
                        rece = sm_pool.tile([64, CW], F32, tag="rece", name="rece")
                        nc.vector.reciprocal(rece[:], bce[:])
                        reco = sm_pool.tile([64, CW], F32, tag="reco", name="reco")
                        nc.vector.reciprocal(reco[:], bco[:])
                        an = avn_pool.tile([P, CW], F32R, tag=f"avn{pair}",
                                           name=f"avn{pair}")
                        nc.vector.tensor_tensor(an[0:64, :], ane[0:64, :],
                                                rece[:], ALU.mult)
                        tmo = sm_pool.tile([64, CW], F32R, tag="tmo", name="tmo")
                        nc.vector.tensor_tensor(tmo[:], ano[0:64, :],
                                                reco[:], ALU.mult)
                        # odd head lands at partitions 64:128 via DMA partition shift
                        nc.sync.dma_start(out=an[64:128, :], in_=tmo[:])
                        avn[pair] = an
                # Wo for this slot -> oT_dram (bias bo folded into eviction)
                for m in range(KC):
                    wom = wstream(wop, Wo_d, m, "wom")
                    ps = pS.tile([P, CW], F32, tag="st", name="wops")
                    for k in range(NPAIR):
                        nc.tensor.matmul(ps[:], r32(wom[:, k, :]),
                                         r32(avn[k][:]), start=(k == 0),
                                         stop=(k == NPAIR - 1))
                    ot = ost_pool.tile([P, CW], F32, tag="ot", name="ot")
                    nc.vector.tensor_scalar(ot[:], ps[:], bot[:, m:m + 1], None,
                                            ALU.add)
                    nc.sync.dma_start(out=oT_dram[m * P:(m + 1) * P, qs], in_=ot[:])

        # =====================================================================
        # Phase 4: x2 = oT + xTq ; LN2 (not in place)
        # =====================================================================
        p_x2t = top.enter_context(tc.tile_pool(name="p_x2t", bufs=1))
        p_ln2T = top.enter_context(tc.tile_pool(name="p_ln2T", bufs=1))
        x2T = []
        ln2T_tiles = []
        with ExitStack() as ph4:
            stream = ph4.enter_context(tc.tile_pool(name="res_stream", bufs=3))
            for kc in range(KC):
                ot = stream.tile([P, NQ], F32, tag="ot_in", name="ot_in")
                nc.sync.dma_start(out=ot[:], in_=oT_dram[kc * P:(kc + 1) * P, :])
                xq = stream.tile([P, NQ], F32R, tag="xq_in", name="xq_in")
                nc.sync.dma_start(out=xq[:], in_=xTq_d[kc * P:(kc + 1) * P, :])
                x2 = p_x2t.tile([P, NQ], F32R, tag=f"x2t{kc}", name=f"x2t{kc}")
                nc.vector.tensor_tensor(x2[:], ot[:], xq[:], ALU.add)
                x2T.append(x2)
                lt = p_ln2T.tile([P, NQ], F32R, tag=f"ln2T{kc}", name=f"ln2T{kc}")
                ln2T_tiles.append(lt)
            ln2T = layernorm_T(x2T, NQ, "g2", "be2", "ln2T", out_tiles=ln2T_tiles)

        # =====================================================================
        # Phase 5: FFN in two hidden-dim halves
        # =====================================================================
        with ExitStack() as ph5:
            ff1_pool = ph5.enter_context(tc.tile_pool(name="ff1", bufs=1))
            facc_pool = ph5.enter_context(tc.tile_pool(name="facc", bufs=1))
            w1_pool = ph5.enter_context(tc.tile_pool(name="w1s", bufs=2))
            w2_pool = ph5.enter_context(tc.tile_pool(name="w2s", bufs=2))
            fst_pool = ph5.enter_context(tc.tile_pool(name="fst", bufs=3))
            ffacc = [facc_pool.tile([P, NQ], F32, tag=f"facc{m}", name=f"ffacc{m}")
                     for m in range(KC)]
            for half in range(2):
                hoff = half * 2048
                ff1 = []
                for m in range(16):
                    mm = half * 16 + m
                    w1m = w1_pool.tile([P, KC, P], F32R, tag="w1m", name="w1m")
                    nc.sync.dma_start(
                        out=w1m[:],
                        in_=W1_d[:, hoff + m * P: hoff + (m + 1) * P]
                        .rearrange("(kc p) m -> p kc m", p=P))
                    f = ff1_pool.tile([P, NQ], F32R, tag=f"f{m}", name=f"f{m}")
                    for tch in range(2):
                        s = slice(tch * 512, (tch + 1) * 512)
                        ps = pA.tile([P, 512], F32, tag="proj", name="ps")
                        for kc in range(KC):
                            nc.tensor.matmul(ps[:], r32(w1m[:, kc, :]),
                                             r32(ln2T[kc][:, s]),
                                             start=(kc == 0), stop=(kc == KC - 1))
                        # relu(x + b1) eviction
                        nc.vector.tensor_scalar(f[:, s], ps[:], b1t[:, mm:mm + 1],
                                                0.0, ALU.add, ALU.max)
                    ff1.append(f)
                for mc in range(KC):
                    w2m = w2_pool.tile([P, 16, P], F32R, tag="w2m", name="w2m")
                    nc.sync.dma_start(
                        out=w2m[:],
                        in_=W2_d[hoff:hoff + 2048, mc * P:(mc + 1) * P]
                        .rearrange("(kt p) m -> p kt m", p=P))
                    for tch in range(2):
                        s = slice(tch * 512, (tch + 1) * 512)
                        ps = pA.tile([P, 512], F32, tag="proj", name="ps")
                        for kt in range(16):
                            nc.tensor.matmul(ps[:], r32(w2m[:, kt, :]),
                                             r32(ff1[kt][:, s]),
                                             start=(kt == 0), stop=(kt == 15))
                        if half == 0:
                            nc.vector.tensor_copy(ffacc[mc][:, s], ps[:])
                        else:
                            o = fst_pool.tile([P, 512], F32, tag="fo", name="fo")
                            nc.vector.tensor_scalar(o[:], ps[:], b2t[:, mc:mc + 1],
                                                    None, ALU.add)
                            nc.vector.tensor_tensor(o[:], o[:], ffacc[mc][:, s],
                                                    ALU.add)
                            nc.vector.tensor_tensor(o[:], o[:], x2T[mc][:, s],
                                                    ALU.add)
                            nc.sync.dma_start(out=out_d[mc * P:(mc + 1) * P, s],
                                              in_=o[:])

    nc.compile()
    return nc, names


_CACHE = {}


def _get_built():
    if "nc" not in _CACHE:
        _CACHE["nc"], _CACHE["names"] = _build()
    return _CACHE["nc"], _CACHE["names"]


def _host_inputs(x, Wq, Wk, Wv, Wo, bo, ln1_g, ln1_b, ln2_g, ln2_b, W1, b1, W2, b2):
    """Build the 8 per-core input maps (host work = sharding/layout only)."""
    f = np.float32
    shared = {
        "Wq": np.ascontiguousarray(Wq, f), "Wk": np.ascontiguousarray(Wk, f),
        "Wv": np.ascontiguousarray(Wv, f), "Wo": np.ascontiguousarray(Wo, f),
        "W1": np.ascontiguousarray(W1, f), "W2": np.ascontiguousarray(W2, f),
        "bot": np.ascontiguousarray(bo.reshape(KC, P).T, f),
        "b1t": np.ascontiguousarray(b1.reshape(32, P).T, f),
        "b2t": np.ascontiguousarray(b2.reshape(KC, P).T, f),
        "g1c": np.ascontiguousarray(ln1_g.reshape(KC, P).T, f),
        "be1c": np.ascontiguousarray(ln1_b.reshape(KC, P).T, f),
        "g2c": np.ascontiguousarray(ln2_g.reshape(KC, P).T, f),
        "be2c": np.ascontiguousarray(ln2_b.reshape(KC, P).T, f),
        "onesC": np.ones((P, 1), f),
        "onesP": np.ones((P, P), f),
    }
    kl = np.arange(P)[:, None]
    ql = np.arange(CW)[None, :]
    in_maps = []
    for c in range(8):
        b, r = c // 2, c % 2
        gs = GSETS[r]
        xTb = np.ascontiguousarray(x[b].T.astype(f))  # (C, T)
        qcols = np.concatenate([np.arange(CW * g, CW * (g + 1)) for g in gs])
        xTq = np.ascontiguousarray(xTb[:, qcols])
        m = np.empty((NSLOT, 4, P, CW), f)
        for s in range(NSLOT):
            q0 = CW * gs[s]
            for j in range(4):
                kt = NKT_PROG[s] - 4 + j
                m[s, j] = ((P * kt + kl) <= (q0 + ql)).astype(f)
        im = dict(shared)
        im["xT"] = xTb
        im["xTq"] = xTq
        im["masks"] = m
        in_maps.append(im)
    return in_maps


def _unshard(outs):
    out = np.empty((4, T, C), np.float32)
    for c in range(8):
        b, r = c // 2, c % 2
        oT = outs[c]  # (C, NQ)
        for s, g in enumerate(GSETS[r]):
            out[b, CW * g:CW * (g + 1), :] = oT[:, CW * s:CW * (s + 1)].T
    return out


def kernel(**inputs):
    from concourse.bass_utils import run_bass_kernel_spmd
    from concourse.bass_interp import get_hw_module

    args = {k: np.asarray(v, np.float32) for k, v in inputs.items()}
    in_maps_named = _host_inputs(**args)

    nc, names = _get_built()
    in_maps = [{names[k]: v for k, v in im.items()} for im in in_maps_named]

    hw = get_hw_module(nc.m)
    old = nc.m
    nc.m = hw
    try:
        res = run_bass_kernel_spmd(nc, in_maps, core_ids=list(range(8)))
    finally:
        nc.m = old
    outs = [r[names["out"]] for r in res.results]
    return _unshard(outs)


if __name__ == "__main__":
    import reference
    inp = {k: np.asarray(v) for k, v in reference.setup_inputs().items()}
    got = kernel(**inp)
    exp = np.asarray(reference.reference(**inp))
    err = np.abs(got - exp).max() / np.abs(exp).max()
    print("Relative error:", err)



# revision 7
# speedup vs baseline: 1.4601x; 1.4601x over previous
"""Trainium2 Bass kernel for a transformer MiniBlock (B=4, T=2048, C=1024, 16 heads,
causal attention, 4x FFN), sharded over 8 NeuronCores.

Sharding: core = (batch b=core//2, role r=core%2). Each core runs the full block for
1024 of its batch's 2048 tokens (two 512-token chunks, balanced for causal work:
role 0 owns chunks {0,3}, role 1 owns {1,2}), computing K/V over the full sequence
(no cross-core communication). The program is SPMD-uniform: k-window loop bounds are
per-slot maxima over roles; per-core causal masks (input data) zero the difference.

All tensors are bf16 on-chip (fp32 PSUM accumulation), which doubles effective
SBUF/DMA capacity, enables fast weight loads, and keeps rel-err ~3e-3. Activations
stay channel-major end to end; LN stats / softmax sums / broadcasts use small
ones-matmuls; the attention softmax is computed k-major with a ones-column appended
to V so denominators fall out of the AV matmul. Weights are pre-tiled on the host so
every weight DMA is fully contiguous. V stays resident in SBUF (no DRAM spill).
Even/odd head score matmuls are row-packed (tile_position) to run concurrently, and
exp is a single 1024-wide activation spanning two PSUM banks.
"""
import sys

sys.path.insert(0, "/opt/trn_rl_repo")

import numpy as np
from contextlib import ExitStack

import concourse.bacc as bacc
import concourse.mybir as mybir
import concourse.tile as tile

F32 = mybir.dt.float32
BF = mybir.dt.bfloat16
AF = mybir.ActivationFunctionType
ALU = mybir.AluOpType

P = 128
T = 2048          # full sequence
C = 1024          # embedding
NQ = 1024         # query tokens per core
H4 = 4096         # ffn hidden
NPAIR = 8         # head pairs (2 heads of 64 dims = 128 channels)
KC = C // P       # 8 channel tiles
NSLOT = 2         # 512-token query chunks per core
CW = 512          # chunk width
NKT_PROG = [8, 16]            # k-tiles per slot (program constant, max over roles)
GSETS = [[0, 3], [1, 2]]      # global 512-chunk index per slot, per role
LN_EPS = 1e-5
SCALE = float(64) ** -0.5     # head_size^-0.5 = 0.125


def _build():
    nc = bacc.Bacc(None, target_bir_lowering=False, debug=False)
    names = {}
    with tile.TileContext(nc) as tc, ExitStack() as top:
        dram = top.enter_context(tc.tile_pool(name="io", bufs=1, space="DRAM"))

        def din(name, shape, dt=BF):
            t = dram.tile(shape, dt, kind="ExternalInput", name=f"i_{name}")
            names[name] = t.name
            return t

        xT_d = din("xT", [C, T])
        xTq_d = din("xTq", [C, NQ])
        wq_d = din("wq", [NPAIR, P, KC * P])
        wk_d = din("wk", [NPAIR, P, KC * P])
        wv_d = din("wv", [2, P, KC * 512])
        wo_d = din("wo", [NPAIR, P, KC * P])
        w1_d = din("w1", [32, P, KC * P])
        w2_d = din("w2", [NPAIR, P, 32 * P])
        masks_d = din("masks", [NSLOT, 8, P, CW])
        bot_d = din("bot", [P, KC], F32)
        b1t_d = din("b1t", [P, 32], F32)
        b2t_d = din("b2t", [P, KC], F32)
        g1_d = din("g1c", [P, KC], F32)
        be1_d = din("be1c", [P, KC], F32)
        g2_d = din("g2c", [P, KC], F32)
        be2_d = din("be2c", [P, KC], F32)
        onesC_d = din("onesC", [P, 1])
        onesP_d = din("onesP", [P, P])

        out_d = dram.tile([C, NQ], F32, kind="ExternalOutput", name="o_out")
        names["out"] = out_d.name

        # ---- persistent small sbuf ----
        pers = top.enter_context(tc.tile_pool(name="pers", bufs=1))
        onesC = pers.tile([P, 1], BF, tag="onesC")
        nc.sync.dma_start(out=onesC[:], in_=onesC_d[:])
        onesP = pers.tile([P, P], BF, tag="onesP")
        nc.sync.dma_start(out=onesP[:], in_=onesP_d[:])
        bot = pers.tile([P, KC], F32, tag="bot")
        nc.sync.dma_start(out=bot[:], in_=bot_d[:])
        b1t = pers.tile([P, 32], F32, tag="b1t")
        nc.sync.dma_start(out=b1t[:], in_=b1t_d[:])
        b2t = pers.tile([P, KC], F32, tag="b2t")
        nc.sync.dma_start(out=b2t[:], in_=b2t_d[:])
        lncol = {}
        for nm, d in [("g1", g1_d), ("be1", be1_d), ("g2", g2_d), ("be2", be2_d)]:
            t = pers.tile([P, KC], F32, tag=f"ln_{nm}", name=f"ln_{nm}")
            nc.sync.dma_start(out=t[:], in_=d[:])
            lncol[nm] = t
        zero_col = pers.tile([P, 1], F32, tag="zero_col")
        nc.vector.memset(zero_col[:], 0.0)
        eps_col = pers.tile([P, 1], F32, tag="eps_col")
        nc.vector.memset(eps_col[:], LN_EPS)

        # =====================================================================
        # transposed-layout layernorm, bf16 (in place unless out_tiles given)
        # =====================================================================
        def layernorm_T(xtiles, n, gname, bname, out_tag, out_tiles=None):
            ctx = ExitStack()
            work = ctx.enter_context(tc.tile_pool(name=f"lnw_{out_tag}", bufs=2))
            stat = ctx.enter_context(tc.tile_pool(name=f"lns_{out_tag}", bufs=1))
            pL = ctx.enter_context(tc.tile_pool(name=f"lnp_{out_tag}", bufs=2,
                                                space="PSUM"))
            pB = ctx.enter_context(tc.tile_pool(name=f"lnb_{out_tag}", bufs=2,
                                                space="PSUM"))
            nn = n // 512
            mu_row = stat.tile([1, n], BF, tag="mu_row")
            msq_row = stat.tile([1, n], BF, tag="msq_row")
            for i in range(nn):
                s = slice(i * 512, (i + 1) * 512)
                ps_sum = pL.tile([1, 512], F32, tag="lsum", name="ps_sum")
                ps_sq = pL.tile([1, 512], F32, tag="lsq", name="ps_sq")
                for kc in range(KC):
                    sq = work.tile([P, 512], BF, tag="sq", name="sq")
                    nc.vector.tensor_tensor(sq[:], xtiles[kc][:, s], xtiles[kc][:, s],
                                            ALU.mult)
                    nc.tensor.matmul(ps_sum[:], onesC[:], xtiles[kc][:, s],
                                     start=(kc == 0), stop=(kc == KC - 1),
                                     skip_group_check=True)
                    nc.tensor.matmul(ps_sq[:], onesC[:], sq[:],
                                     start=(kc == 0), stop=(kc == KC - 1),
                                     skip_group_check=True)
                nc.scalar.activation(mu_row[:, s], ps_sum[:], AF.Copy, scale=1.0 / C)
                nc.scalar.activation(msq_row[:, s], ps_sq[:], AF.Copy, scale=1.0 / C)
            mu_b = stat.tile([P, n], BF, tag="mu_b")
            rstd_b = stat.tile([P, n], BF, tag="rstd_b")
            for i in range(nn):
                s = slice(i * 512, (i + 1) * 512)
                psb = pB.tile([P, 1024], F32, tag="bc", name="psb")
                nc.tensor.matmul(psb[:, 0:512], onesP[0:1, :], mu_row[:, s],
                                 start=True, stop=True)
                nc.tensor.matmul(psb[:, 512:1024], onesP[0:1, :], msq_row[:, s],
                                 start=True, stop=True)
                nc.vector.tensor_copy(mu_b[:, s], psb[:, 0:512])
                mu2 = work.tile([P, 512], BF, tag="mu2", name="mu2")
                nc.vector.tensor_tensor(mu2[:], mu_b[:, s], mu_b[:, s], ALU.mult)
                var = work.tile([P, 512], F32, tag="var", name="var")
                nc.vector.tensor_tensor(var[:], psb[:, 512:1024], mu2[:],
                                        ALU.subtract)
                nc.scalar.activation(var[:], var[:], AF.Sqrt, bias=eps_col[:])
                with nc.allow_low_precision(reason="bf16 rstd is plenty for 2e-2"):
                    nc.vector.reciprocal(rstd_b[:, s], var[:])
            outs = []
            for kc in range(KC):
                o = out_tiles[kc] if out_tiles is not None else xtiles[kc]
                nc.vector.tensor_tensor(o[:], xtiles[kc][:], mu_b[:], ALU.subtract)
                nc.vector.tensor_tensor(o[:], o[:], rstd_b[:], ALU.mult)
                nc.vector.tensor_scalar(o[:], o[:], lncol[gname][:, kc:kc + 1],
                                        lncol[bname][:, kc:kc + 1], ALU.mult, ALU.add)
                outs.append(o)
            ctx.close()
            return outs

        # =====================================================================
        # Phase 1: q path — LN1 of the core's own query columns, project
        # =====================================================================
        p_xTq = top.enter_context(tc.tile_pool(name="p_xTq", bufs=1))
        p_oT = top.enter_context(tc.tile_pool(name="p_oT", bufs=1))
        oT = [p_oT.tile([P, NSLOT, CW], BF, tag=f"oT{m}", name=f"oT{m}")
              for m in range(NPAIR)]
        att_ctx = ExitStack()
        p_qT = att_ctx.enter_context(tc.tile_pool(name="p_qT", bufs=1))
        xTq = []
        for kc in range(KC):
            tq = p_xTq.tile([P, NQ], BF, tag=f"xTq{kc}", name=f"xTq{kc}")
            nc.sync.dma_start(out=tq[:], in_=xTq_d[kc * P:(kc + 1) * P, :])
            xTq.append(tq)
        qT = []
        with ExitStack() as phq:
            p_ln1q = phq.enter_context(tc.tile_pool(name="p_ln1q", bufs=1))
            ln1q_tiles = [p_ln1q.tile([P, NQ], BF, tag=f"ln1q{kc}", name=f"ln1q{kc}")
                          for kc in range(KC)]
            ln1q = layernorm_T(xTq, NQ, "g1", "be1", "ln1q", out_tiles=ln1q_tiles)
            wqp = phq.enter_context(tc.tile_pool(name="wqp", bufs=2))
            pA = phq.enter_context(tc.tile_pool(name="pAq", bufs=3, space="PSUM"))
            for m in range(NPAIR):
                wqm = wqp.tile([P, KC, P], BF, tag="wqm", name="wqm")
                nc.sync.dma_start(
                    out=wqm[:], in_=wq_d[m].rearrange("p (k m) -> p k m", k=KC))
                qt = p_qT.tile([P, NQ], BF, tag=f"qT{m}", name=f"qT{m}")
                for nq in range(NQ // 512):
                    s = slice(nq * 512, (nq + 1) * 512)
                    ps = pA.tile([P, 512], F32, tag="proj", name="ps")
                    for kc in range(KC):
                        nc.tensor.matmul(ps[:], wqm[:, kc, :], ln1q[kc][:, s],
                                         start=(kc == 0), stop=(kc == KC - 1))
                    nc.vector.tensor_copy(qt[:, s], ps[:])
                qT.append(qt)

        # =====================================================================
        # Phase 2: k and v paths — LN1 over the full sequence
        # =====================================================================
        p_kT = att_ctx.enter_context(tc.tile_pool(name="p_kT", bufs=1))
        p_v65 = att_ctx.enter_context(tc.tile_pool(name="p_v65", bufs=1))
        v65 = p_v65.tile([P, T // P, NPAIR, 2, 65], BF, tag="v65", name="v65")
        nc.vector.memset(v65[:, :, :, :, 64:65], 1.0)
        kT = []
        with ExitStack() as phk:
            p_xT = phk.enter_context(tc.tile_pool(name="p_xT", bufs=1))
            xT = []
            for kc in range(KC):
                t = p_xT.tile([P, T], BF, tag=f"xT{kc}", name=f"xT{kc}")
                nc.sync.dma_start(out=t[:], in_=xT_d[kc * P:(kc + 1) * P, :])
                xT.append(t)
            ln1T = layernorm_T(xT, T, "g1", "be1", "ln1T")

            with ExitStack() as phkw:
                wkp = phkw.enter_context(tc.tile_pool(name="wkp", bufs=2))
                pA = phkw.enter_context(tc.tile_pool(name="pAk", bufs=3,
                                                     space="PSUM"))
                for m in range(NPAIR):
                    wkm = wkp.tile([P, KC, P], BF, tag="wkm", name="wkm")
                    nc.sync.dma_start(
                        out=wkm[:], in_=wk_d[m].rearrange("p (k m) -> p k m", k=KC))
                    kt_t = p_kT.tile([P, T], BF, tag=f"kT{m}", name=f"kT{m}")
                    for n in range(T // 512):
                        s = slice(n * 512, (n + 1) * 512)
                        ps = pA.tile([P, 512], F32, tag="proj", name="ps")
                        for kc in range(KC):
                            nc.tensor.matmul(ps[:], wkm[:, kc, :], ln1T[kc][:, s],
                                             start=(kc == 0), stop=(kc == KC - 1))
                        nc.vector.tensor_copy(kt_t[:, s], ps[:])
                    kT.append(kt_t)

            # v: token-major into resident v65 (ones col prefilled)
            with ExitStack() as phv:
                wvp = phv.enter_context(tc.tile_pool(name="wvp", bufs=2))
                pA = phv.enter_context(tc.tile_pool(name="pAv", bufs=3,
                                                    space="PSUM"))
                for n in range(2):
                    wvn = wvp.tile([P, KC, 512], BF, tag="wvn", name="wvn")
                    nc.sync.dma_start(
                        out=wvn[:], in_=wv_d[n].rearrange("p (k d) -> p k d", k=KC))
                    for tt in range(T // P):
                        ps = pA.tile([P, 512], F32, tag="proj", name="ps")
                        for kc in range(KC):
                            nc.tensor.matmul(
                                ps[:], ln1T[kc][:, tt * P:(tt + 1) * P],
                                wvn[:, kc, :],
                                start=(kc == 0), stop=(kc == KC - 1))
                        nc.vector.tensor_copy(
                            v65[:, tt, 4 * n:4 * (n + 1), :, 0:64],
                            ps[:].rearrange("p (pr par d) -> p pr par d",
                                            pr=4, par=2))

        # =====================================================================
        # Phase 3: attention (ln1T freed; masks/avn fit above qT/kT/v65)
        # =====================================================================
        with ExitStack() as ph3:
            p_mask = ph3.enter_context(tc.tile_pool(name="p_mask", bufs=1))
            masks = {}
            for s in range(NSLOT):
                for j in range(8):
                    mt = p_mask.tile([P, CW], BF, tag=f"mask{s}_{j}",
                                     name=f"mask{s}_{j}")
                    nc.sync.dma_start(out=mt[:], in_=masks_d[s, j])
                    masks[(s, j)] = mt
            avn_pool = ph3.enter_context(tc.tile_pool(name="avn", bufs=2))
            sm_pool = ph3.enter_context(tc.tile_pool(name="sm", bufs=3))
            pt_pool = ph3.enter_context(tc.tile_pool(name="pt", bufs=4))
            wop = ph3.enter_context(tc.tile_pool(name="wop", bufs=2))

            for s in range(NSLOT):
                nkt = NKT_PROG[s]
                mask_base = 0 if s == 0 else 8
                qs = slice(s * CW, (s + 1) * CW)
                avn = {}
                with ExitStack() as phs:
                    psc_pool = phs.enter_context(
                        tc.tile_pool(name="psc", bufs=2, space="PSUM"))
                    pav_pool = phs.enter_context(
                        tc.tile_pool(name="pav", bufs=1, space="PSUM"))
                    pbc_pool = phs.enter_context(
                        tc.tile_pool(name="pbc", bufs=1, space="PSUM"))
                    for pair in range(NPAIR):
                        av = pav_pool.tile([65, 1024], F32, tag="av", name="av")
                        for kt in range(nkt):
                            kws = slice(kt * P, (kt + 1) * P)
                            psc = psc_pool.tile([P, 1024], F32, tag="sc",
                                                name="psc")
                            nc.tensor.matmul(psc[:, 0:512], kT[pair][0:64, kws],
                                             qT[pair][0:64, qs],
                                             start=True, stop=True)
                            nc.tensor.matmul(psc[:, 512:1024],
                                             kT[pair][64:128, kws],
                                             qT[pair][64:128, qs],
                                             start=True, stop=True)
                            pt = pt_pool.tile([P, 1024], BF, tag="pt", name="pt")
                            nc.scalar.activation(pt[:], psc[:], AF.Exp,
                                                 bias=zero_col[:], scale=SCALE)
                            jm = kt - mask_base
                            if 0 <= jm < 8:
                                mt = masks[(s, jm)]
                                nc.vector.tensor_tensor(pt[:, 0:512], pt[:, 0:512],
                                                        mt[:], ALU.mult)
                                nc.vector.tensor_tensor(pt[:, 512:1024],
                                                        pt[:, 512:1024],
                                                        mt[:], ALU.mult)
                            st = (kt == 0)
                            sp = (kt == nkt - 1)
                            nc.tensor.matmul(av[0:65, 0:512],
                                             v65[:, kt, pair, 0, :],
                                             pt[:, 0:512],
                                             start=st, stop=sp,
                                             skip_group_check=True)
                            nc.tensor.matmul(av[0:65, 512:1024],
                                             v65[:, kt, pair, 1, :],
                                             pt[:, 512:1024],
                                             start=st, stop=sp,
                                             skip_group_check=True)
                        # normalize: evict, broadcast sums, reciprocal, scale
                        an = sm_pool.tile([65, 1024], BF, tag="an", name="an")
                        nc.vector.tensor_copy(an[:], av[0:65, :])
                        bc = pbc_pool.tile([64, 1024], F32, tag="bc", name="bc")
                        nc.tensor.matmul(bc[:, 0:512], onesP[64:65, 0:64],
                                         an[64:65, 0:512], start=True, stop=True)
                        nc.tensor.matmul(bc[:, 512:1024], onesP[64:65, 0:64],
                                         an[64:65, 512:1024], start=True, stop=True)
                        rec = sm_pool.tile([64, 1024], BF, tag="rec", name="rec")
                        with nc.allow_low_precision(reason="bf16 softmax denom"):
                            nc.vector.reciprocal(rec[:], bc[:])
                        anp = avn_pool.tile([P, CW], BF, tag=f"avn{pair}",
                                            name=f"avn{pair}")
                        nc.vector.tensor_tensor(anp[0:64, :], an[0:64, 0:512],
                                                rec[:, 0:512], ALU.mult)
                        tmo = sm_pool.tile([64, CW], BF, tag="tmo", name="tmo")
                        nc.vector.tensor_tensor(tmo[:], an[0:64, 512:1024],
                                                rec[:, 512:1024], ALU.mult)
                        nc.sync.dma_start(out=anp[64:128, :], in_=tmo[:])
                        avn[pair] = anp
                # Wo for this slot (bias bo folded into eviction)
                with ExitStack() as phwo:
                    pwo = phwo.enter_context(
                        tc.tile_pool(name="pwo", bufs=2, space="PSUM"))
                    for m in range(NPAIR):
                        wom = wop.tile([P, KC, P], BF, tag="wom", name="wom")
                        nc.sync.dma_start(
                            out=wom[:],
                            in_=wo_d[m].rearrange("p (k m) -> p k m", k=KC))
                        ps = pwo.tile([P, CW], F32, tag="wops", name="wops")
                        for k in range(NPAIR):
                            nc.tensor.matmul(ps[:], wom[:, k, :], avn[k][:],
                                             start=(k == 0), stop=(k == NPAIR - 1))
                        nc.vector.tensor_scalar(oT[m][:, s, :], ps[:],
                                                bot[:, m:m + 1], None, ALU.add)

        att_ctx.close()

        # =====================================================================
        # Phase 4: x2 = oT + xTq ; LN2 (not in place)
        # =====================================================================
        p_x2t = top.enter_context(tc.tile_pool(name="p_x2t", bufs=1))
        p_ln2T = top.enter_context(tc.tile_pool(name="p_ln2T", bufs=1))
        x2T = []
        ln2T_tiles = []
        for kc in range(KC):
            x2 = p_x2t.tile([P, NQ], BF, tag=f"x2t{kc}", name=f"x2t{kc}")
            nc.vector.tensor_tensor(
                x2[:], xTq[kc][:],
                oT[kc][:].rearrange("p s w -> p (s w)"), ALU.add)
            x2T.append(x2)
            lt = p_ln2T.tile([P, NQ], BF, tag=f"ln2T{kc}", name=f"ln2T{kc}")
            ln2T_tiles.append(lt)
        ln2T = layernorm_T(x2T, NQ, "g2", "be2", "ln2T", out_tiles=ln2T_tiles)

        # =====================================================================
        # Phase 5: FFN in two hidden-dim halves
        # =====================================================================
        with ExitStack() as ph5:
            ff1_pool = ph5.enter_context(tc.tile_pool(name="ff1", bufs=1))
            facc_pool = ph5.enter_context(tc.tile_pool(name="facc", bufs=1))
            w1_pool = ph5.enter_context(tc.tile_pool(name="w1s", bufs=2))
            w2_pool = ph5.enter_context(tc.tile_pool(name="w2s", bufs=2))
            fst_pool = ph5.enter_context(tc.tile_pool(name="fst", bufs=3))
            pF = ph5.enter_context(tc.tile_pool(name="pF", bufs=4, space="PSUM"))
            ffacc = [facc_pool.tile([P, NQ], BF, tag=f"facc{m}", name=f"ffacc{m}")
                     for m in range(KC)]
            for half in range(2):
                ff1 = []
                for m in range(16):
                    mm = half * 16 + m
                    w1m = w1_pool.tile([P, KC, P], BF, tag="w1m", name="w1m")
                    nc.sync.dma_start(
                        out=w1m[:],
                        in_=w1_d[mm].rearrange("p (k m) -> p k m", k=KC))
                    f = ff1_pool.tile([P, NQ], BF, tag=f"f{m}", name=f"f{m}")
                    for tch in range(2):
                        s = slice(tch * 512, (tch + 1) * 512)
                        ps = pF.tile([P, 512], F32, tag="proj", name="ps")
                        for kc in range(KC):
                            nc.tensor.matmul(ps[:], w1m[:, kc, :], ln2T[kc][:, s],
                                             start=(kc == 0), stop=(kc == KC - 1))
                        # relu(x + b1) eviction
                        nc.vector.tensor_scalar(f[:, s], ps[:], b1t[:, mm:mm + 1],
                                                0.0, ALU.add, ALU.max)
                    ff1.append(f)
                for mc in range(KC):
                    w2m = w2_pool.tile([P, 16, P], BF, tag="w2m", name="w2m")
                    nc.sync.dma_start(
                        out=w2m[:],
                        in_=w2_d[mc][:, half * 2048:(half + 1) * 2048]
                        .rearrange("p (k m) -> p k m", k=16))
                    for tch in range(2):
                        s = slice(tch * 512, (tch + 1) * 512)
                        ps = pF.tile([P, 512], F32, tag="proj", name="ps")
                        for kt in range(16):
                            nc.tensor.matmul(ps[:], w2m[:, kt, :], ff1[kt][:, s],
                                             start=(kt == 0), stop=(kt == 15))
                        if half == 0:
                            nc.vector.tensor_copy(ffacc[mc][:, s], ps[:])
                        else:
                            o = fst_pool.tile([P, 512], F32, tag="fo", name="fo")
                            nc.vector.tensor_scalar(o[:], ps[:], b2t[:, mc:mc + 1],
                                                    None, ALU.add)
                            nc.vector.tensor_tensor(o[:], o[:], ffacc[mc][:, s],
                                                    ALU.add)
                            nc.vector.tensor_tensor(o[:], o[:], x2T[mc][:, s],
                                                    ALU.add)
                            nc.sync.dma_start(out=out_d[mc * P:(mc + 1) * P, s],
                                              in_=o[:])

    nc.compile()
    return nc, names


_CACHE = {}


def _get_built():
    if "nc" not in _CACHE:
        _CACHE["nc"], _CACHE["names"] = _build()
    return _CACHE["nc"], _CACHE["names"]


def _host_inputs(x, Wq, Wk, Wv, Wo, bo, ln1_g, ln1_b, ln2_g, ln2_b, W1, b1, W2, b2):
    """Build the 8 per-core input maps (host work = sharding/layout only)."""
    from ml_dtypes import bfloat16
    f = np.float32

    def wtile(W, nmb, nkc):
        # [mb, p, kc*P_or_512] with [mb,p,kc*w+j] = W[kc*P+p, mb*wout+j]
        kin, cout = W.shape
        wout = cout // nmb
        return np.ascontiguousarray(
            W.reshape(nkc, P, nmb, wout).transpose(2, 1, 0, 3)
            .reshape(nmb, P, nkc * wout).astype(bfloat16))

    shared = {
        "wq": wtile(np.asarray(Wq, f), NPAIR, KC),
        "wk": wtile(np.asarray(Wk, f), NPAIR, KC),
        "wv": wtile(np.asarray(Wv, f), 2, KC),
        "wo": wtile(np.asarray(Wo, f), NPAIR, KC),
        "w1": wtile(np.asarray(W1, f), 32, KC),
        "w2": wtile(np.asarray(W2, f), NPAIR, 32),
        "bot": np.ascontiguousarray(np.asarray(bo, f).reshape(KC, P).T),
        "b1t": np.ascontiguousarray(np.asarray(b1, f).reshape(32, P).T),
        "b2t": np.ascontiguousarray(np.asarray(b2, f).reshape(KC, P).T),
        "g1c": np.ascontiguousarray(np.asarray(ln1_g, f).reshape(KC, P).T),
        "be1c": np.ascontiguousarray(np.asarray(ln1_b, f).reshape(KC, P).T),
        "g2c": np.ascontiguousarray(np.asarray(ln2_g, f).reshape(KC, P).T),
        "be2c": np.ascontiguousarray(np.asarray(ln2_b, f).reshape(KC, P).T),
        "onesC": np.ones((P, 1), bfloat16),
        "onesP": np.ones((P, P), bfloat16),
    }
    kl = np.arange(P)[:, None]
    ql = np.arange(CW)[None, :]
    in_maps = []
    for c in range(8):
        b, r = c // 2, c % 2
        gs = GSETS[r]
        xTb = np.ascontiguousarray(np.asarray(x[b], f).T.astype(bfloat16))
        qcols = np.concatenate([np.arange(CW * g, CW * (g + 1)) for g in gs])
        xTq = np.ascontiguousarray(xTb[:, qcols])
        m = np.empty((NSLOT, 8, P, CW), bfloat16)
        for s in range(NSLOT):
            q0 = CW * gs[s]
            base = 0 if s == 0 else 8
            for j in range(8):
                kt = base + j
                m[s, j] = ((P * kt + kl) <= (q0 + ql)).astype(bfloat16)
        im = dict(shared)
        im["xT"] = xTb
        im["xTq"] = xTq
        im["masks"] = m
        in_maps.append(im)
    return in_maps


def _unshard(outs):
    out = np.empty((4, T, C), np.float32)
    for c in range(8):
        b, r = c // 2, c % 2
        oT = outs[c]  # (C, NQ)
        for s, g in enumerate(GSETS[r]):
            out[b, CW * g:CW * (g + 1), :] = oT[:, CW * s:CW * (s + 1)].T
    return out


def kernel(**inputs):
    from concourse.bass_utils import run_bass_kernel_spmd
    from concourse.bass_interp import get_hw_module

    args = {k: np.asarray(v, np.float32) for k, v in inputs.items()}
    in_maps_named = _host_inputs(**args)

    nc, names = _get_built()
    in_maps = [{names[k]: v for k, v in im.items()} for im in in_maps_named]

    hw = get_hw_module(nc.m)
    old = nc.m
    nc.m = hw
    try:
        res = run_bass_kernel_spmd(nc, in_maps, core_ids=list(range(8)))
    finally:
        nc.m = old
    outs = [r[names["out"]] for r in res.results]
    return _unshard(outs)


if __name__ == "__main__":
    import reference
    inp = {k: np.asarray(v) for k, v in reference.setup_inputs().items()}
    got = kernel(**inp)
    exp = np.asarray(reference.reference(**inp))
    err = np.abs(got - exp).max() / np.abs(exp).max()
    print("Relative error:", err)


# revision 11
# speedup vs baseline: 1.5587x; 1.0675x over previous
"""Trainium2 Bass kernel for a transformer MiniBlock (B=4, T=2048, C=1024, 16 heads,
causal attention, 4x FFN), sharded over 8 NeuronCores.

Sharding: core = (batch b=core//2, role r=core%2). Each core runs the full block for
1024 of its batch's 2048 tokens (two 512-token chunks, balanced for causal work:
role 0 owns chunks {0,3}, role 1 owns {1,2}), computing K/V over the full sequence
(no cross-core communication). The program is SPMD-uniform: k-window loop bounds are
per-slot maxima over roles; per-core causal masks (input data) zero the difference.

All tensors are bf16 on-chip (fp32 PSUM accumulation), which doubles effective
SBUF/DMA capacity, enables fast weight loads, and keeps rel-err ~3e-3. Activations
stay channel-major end to end; LN stats / softmax sums / broadcasts use small
ones-matmuls; the attention softmax is computed k-major with a ones-column appended
to V so denominators fall out of the AV matmul. Weights are pre-tiled on the host so
every weight DMA is fully contiguous. V stays resident in SBUF (no DRAM spill).
Even/odd head score matmuls are row-packed (tile_position) to run concurrently, and
exp is a single 1024-wide activation spanning two PSUM banks.
"""
import sys

sys.path.insert(0, "/opt/trn_rl_repo")

import numpy as np
from contextlib import ExitStack

import concourse.bacc as bacc
import concourse.mybir as mybir
import concourse.tile as tile

F32 = mybir.dt.float32
BF = mybir.dt.bfloat16
AF = mybir.ActivationFunctionType
ALU = mybir.AluOpType

P = 128
T = 2048          # full sequence
C = 1024          # embedding
NQ = 1024         # query tokens per core
H4 = 4096         # ffn hidden
NPAIR = 8         # head pairs (2 heads of 64 dims = 128 channels)
KC = C // P       # 8 channel tiles
NSLOT = 2         # 512-token query chunks per core
CW = 512          # chunk width
NKT_PROG = [8, 16]            # k-tiles per slot (program constant, max over roles)
GSETS = [[0, 3], [1, 2]]      # global 512-chunk index per slot, per role
LN_EPS = 1e-5
SCALE = float(64) ** -0.5     # head_size^-0.5 = 0.125


def _build():
    nc = bacc.Bacc(None, target_bir_lowering=False, debug=False)
    names = {}
    with tile.TileContext(nc) as tc, ExitStack() as top:
        dram = top.enter_context(tc.tile_pool(name="io", bufs=1, space="DRAM"))

        def din(name, shape, dt=BF):
            t = dram.tile(shape, dt, kind="ExternalInput", name=f"i_{name}")
            names[name] = t.name
            return t

        xT_d = din("xT", [C, T])
        xTq_d = din("xTq", [C, NQ])
        wq_d = din("wq", [NPAIR, P, KC * P])
        wk_d = din("wk", [NPAIR, P, KC * P])
        wv_d = din("wv", [2, P, KC * 512])
        wo_d = din("wo", [NPAIR, P, KC * P])
        w1_d = din("w1", [32, P, KC * P])
        w2_d = din("w2", [NPAIR, P, 32 * P])
        masks_d = din("masks", [NSLOT, 8, P, CW])
        bot_d = din("bot", [P, KC], F32)
        b1t_d = din("b1t", [P, 32], F32)
        b2t_d = din("b2t", [P, KC], F32)
        g1_d = din("g1c", [P, KC], F32)
        be1_d = din("be1c", [P, KC], F32)
        g2_d = din("g2c", [P, KC], F32)
        be2_d = din("be2c", [P, KC], F32)
        onesC_d = din("onesC", [P, 1])
        onesP_d = din("onesP", [P, P])

        out_d = dram.tile([C, NQ], F32, kind="ExternalOutput", name="o_out")
        names["out"] = out_d.name

        # ---- persistent small sbuf ----
        pers = top.enter_context(tc.tile_pool(name="pers", bufs=1))
        onesC = pers.tile([P, 1], BF, tag="onesC")
        nc.sync.dma_start(out=onesC[:], in_=onesC_d[:])
        onesP = pers.tile([P, P], BF, tag="onesP")
        nc.sync.dma_start(out=onesP[:], in_=onesP_d[:])
        bot = pers.tile([P, KC], F32, tag="bot")
        nc.sync.dma_start(out=bot[:], in_=bot_d[:])
        b1t = pers.tile([P, 32], F32, tag="b1t")
        nc.sync.dma_start(out=b1t[:], in_=b1t_d[:])
        b2t = pers.tile([P, KC], F32, tag="b2t")
        nc.sync.dma_start(out=b2t[:], in_=b2t_d[:])
        lncol = {}
        for nm, d in [("g1", g1_d), ("be1", be1_d), ("g2", g2_d), ("be2", be2_d)]:
            t = pers.tile([P, KC], F32, tag=f"ln_{nm}", name=f"ln_{nm}")
            nc.sync.dma_start(out=t[:], in_=d[:])
            lncol[nm] = t
        zero_col = pers.tile([P, 1], F32, tag="zero_col")
        nc.vector.memset(zero_col[:], 0.0)
        eps_col = pers.tile([P, 1], F32, tag="eps_col")
        nc.vector.memset(eps_col[:], LN_EPS)

        # =====================================================================
        # transposed-layout layernorm, bf16 (in place unless out_tiles given)
        # =====================================================================
        def layernorm_T(xtiles, n, gname, bname, out_tag, out_tiles=None):
            ctx = ExitStack()
            work = ctx.enter_context(tc.tile_pool(name=f"lnw_{out_tag}", bufs=2))
            stat = ctx.enter_context(tc.tile_pool(name=f"lns_{out_tag}", bufs=1))
            pL = ctx.enter_context(tc.tile_pool(name=f"lnp_{out_tag}", bufs=2,
                                                space="PSUM"))
            pB = ctx.enter_context(tc.tile_pool(name=f"lnb_{out_tag}", bufs=2,
                                                space="PSUM"))
            nn = n // 512
            mu_row = stat.tile([1, n], BF, tag="mu_row")
            msq_row = stat.tile([1, n], BF, tag="msq_row")
            for i in range(nn):
                s = slice(i * 512, (i + 1) * 512)
                ps_sum = pL.tile([1, 512], F32, tag="lsum", name="ps_sum")
                ps_sq = pL.tile([1, 512], F32, tag="lsq", name="ps_sq")
                for kc in range(KC):
                    sq = work.tile([P, 512], BF, tag="sq", name="sq")
                    nc.vector.tensor_tensor(sq[:], xtiles[kc][:, s], xtiles[kc][:, s],
                                            ALU.mult)
                    nc.tensor.matmul(ps_sum[:], onesC[:], xtiles[kc][:, s],
                                     start=(kc == 0), stop=(kc == KC - 1),
                                     skip_group_check=True)
                    nc.tensor.matmul(ps_sq[:], onesC[:], sq[:],
                                     start=(kc == 0), stop=(kc == KC - 1),
                                     skip_group_check=True)
                nc.scalar.activation(mu_row[:, s], ps_sum[:], AF.Copy, scale=1.0 / C)
                nc.scalar.activation(msq_row[:, s], ps_sq[:], AF.Copy, scale=1.0 / C)
            mu_b = stat.tile([P, n], BF, tag="mu_b")
            rstd_b = stat.tile([P, n], BF, tag="rstd_b")
            for i in range(nn):
                s = slice(i * 512, (i + 1) * 512)
                psb = pB.tile([P, 1024], F32, tag="bc", name="psb")
                nc.tensor.matmul(psb[:, 0:512], onesP[0:1, :], mu_row[:, s],
                                 start=True, stop=True)
                nc.tensor.matmul(psb[:, 512:1024], onesP[0:1, :], msq_row[:, s],
                                 start=True, stop=True)
                nc.vector.tensor_copy(mu_b[:, s], psb[:, 0:512])
                mu2 = work.tile([P, 512], BF, tag="mu2", name="mu2")
                nc.vector.tensor_tensor(mu2[:], mu_b[:, s], mu_b[:, s], ALU.mult)
                var = work.tile([P, 512], F32, tag="var", name="var")
                nc.vector.tensor_tensor(var[:], psb[:, 512:1024], mu2[:],
                                        ALU.subtract)
                nc.scalar.activation(var[:], var[:], AF.Ln, bias=eps_col[:])
                nc.scalar.activation(rstd_b[:, s], var[:], AF.Exp, scale=-0.5)
            outs = []
            for kc in range(KC):
                o = out_tiles[kc] if out_tiles is not None else xtiles[kc]
                nc.vector.tensor_tensor(o[:], xtiles[kc][:], mu_b[:], ALU.subtract)
                nc.vector.tensor_tensor(o[:], o[:], rstd_b[:], ALU.mult)
                nc.vector.tensor_scalar(o[:], o[:], lncol[gname][:, kc:kc + 1],
                                        lncol[bname][:, kc:kc + 1], ALU.mult, ALU.add)
                outs.append(o)
            ctx.close()
            return outs

        # =====================================================================
        # Phase 1: q path — LN1 of the core's own query columns, project
        # =====================================================================
        p_xTq = top.enter_context(tc.tile_pool(name="p_xTq", bufs=1))
        p_oT = top.enter_context(tc.tile_pool(name="p_oT", bufs=1))
        oT = [p_oT.tile([P, NSLOT, CW], BF, tag=f"oT{m}", name=f"oT{m}")
              for m in range(NPAIR)]
        att_ctx = ExitStack()
        p_qT = att_ctx.enter_context(tc.tile_pool(name="p_qT", bufs=1))
        xTq = []
        for kc in range(KC):
            tq = p_xTq.tile([P, NQ], BF, tag=f"xTq{kc}", name=f"xTq{kc}")
            nc.sync.dma_start(out=tq[:], in_=xTq_d[kc * P:(kc + 1) * P, :])
            xTq.append(tq)
        qT = []
        p_ln1q = att_ctx.enter_context(tc.tile_pool(name="p_ln1q", bufs=1))
        ln1q_tiles = [p_ln1q.tile([P, NQ], BF, tag=f"ln1q{kc}", name=f"ln1q{kc}")
                      for kc in range(KC)]
        ln1q = layernorm_T(xTq, NQ, "g1", "be1", "ln1q", out_tiles=ln1q_tiles)

        # =====================================================================
        # Phase 2: k and v paths — LN1 over the full sequence (Q projection is
        # issued right after LN1T so its matmuls overlap the LN1T apply ops)
        # =====================================================================
        p_kT = att_ctx.enter_context(tc.tile_pool(name="p_kT", bufs=1))
        p_v65 = att_ctx.enter_context(tc.tile_pool(name="p_v65", bufs=1))
        v65 = p_v65.tile([P, T // P, NPAIR, 2, 65], BF, tag="v65", name="v65")
        nc.vector.memset(v65[:, :, :, :, 64:65], 1.0)
        kT = []
        with ExitStack() as phk:
            p_xT = phk.enter_context(tc.tile_pool(name="p_xT", bufs=1))
            xT = []
            for kc in range(KC):
                t = p_xT.tile([P, T], BF, tag=f"xT{kc}", name=f"xT{kc}")
                nc.sync.dma_start(out=t[:], in_=xT_d[kc * P:(kc + 1) * P, :])
                xT.append(t)
            ln1T = layernorm_T(xT, T, "g1", "be1", "ln1T")

            with ExitStack() as phqw:
                wqp = phqw.enter_context(tc.tile_pool(name="wqp", bufs=2))
                pA = phqw.enter_context(tc.tile_pool(name="pAq", bufs=3,
                                                     space="PSUM"))
                for m in range(NPAIR):
                    wqm = wqp.tile([P, KC, P], BF, tag="wqm", name="wqm")
                    nc.sync.dma_start(
                        out=wqm[:], in_=wq_d[m].rearrange("p (k m) -> p k m", k=KC))
                    qt = p_qT.tile([P, NQ], BF, tag=f"qT{m}", name=f"qT{m}")
                    for nq in range(NQ // 512):
                        s = slice(nq * 512, (nq + 1) * 512)
                        ps = pA.tile([P, 512], F32, tag="proj", name="ps")
                        for kc in range(KC):
                            nc.tensor.matmul(ps[:], wqm[:, kc, :], ln1q[kc][:, s],
                                             start=(kc == 0), stop=(kc == KC - 1))
                        nc.scalar.activation(qt[:, s], ps[:], AF.Copy)
                    qT.append(qt)

            with ExitStack() as phkw:
                wkp = phkw.enter_context(tc.tile_pool(name="wkp", bufs=2))
                pA = phkw.enter_context(tc.tile_pool(name="pAk", bufs=3,
                                                     space="PSUM"))
                for m in range(NPAIR):
                    wkm = wkp.tile([P, KC, P], BF, tag="wkm", name="wkm")
                    nc.sync.dma_start(
                        out=wkm[:], in_=wk_d[m].rearrange("p (k m) -> p k m", k=KC))
                    kt_t = p_kT.tile([P, T], BF, tag=f"kT{m}", name=f"kT{m}")
                    for n in range(T // 512):
                        s = slice(n * 512, (n + 1) * 512)
                        ps = pA.tile([P, 512], F32, tag="proj", name="ps")
                        for kc in range(KC):
                            nc.tensor.matmul(ps[:], wkm[:, kc, :], ln1T[kc][:, s],
                                             start=(kc == 0), stop=(kc == KC - 1))
                        nc.scalar.activation(kt_t[:, s], ps[:], AF.Copy)
                    kT.append(kt_t)

            # v: token-major into resident v65 (ones col prefilled)
            with ExitStack() as phv:
                wvp = phv.enter_context(tc.tile_pool(name="wvp", bufs=2))
                pA = phv.enter_context(tc.tile_pool(name="pAv", bufs=3,
                                                    space="PSUM"))
                for n in range(2):
                    wvn = wvp.tile([P, KC, 512], BF, tag="wvn", name="wvn")
                    nc.sync.dma_start(
                        out=wvn[:], in_=wv_d[n].rearrange("p (k d) -> p k d", k=KC))
                    for tt in range(T // P):
                        ps = pA.tile([P, 512], F32, tag="proj", name="ps")
                        for kc in range(KC):
                            nc.tensor.matmul(
                                ps[:], ln1T[kc][:, tt * P:(tt + 1) * P],
                                wvn[:, kc, :],
                                start=(kc == 0), stop=(kc == KC - 1))
                        nc.vector.tensor_copy(
                            v65[:, tt, 4 * n:4 * (n + 1), :, 0:64],
                            ps[:].rearrange("p (pr par d) -> p pr par d",
                                            pr=4, par=2))

        # =====================================================================
        # Phase 3: attention (ln1T freed; masks/avn fit above qT/kT/v65)
        # =====================================================================
        with ExitStack() as ph3:
            p_mask = ph3.enter_context(tc.tile_pool(name="p_mask", bufs=1))
            masks = {}
            for s in range(NSLOT):
                for j in range(8):
                    mt = p_mask.tile([P, CW], BF, tag=f"mask{s}_{j}",
                                     name=f"mask{s}_{j}")
                    nc.sync.dma_start(out=mt[:], in_=masks_d[s, j])
                    masks[(s, j)] = mt
            avn_pool = ph3.enter_context(tc.tile_pool(name="avn", bufs=2))
            sm_pool = ph3.enter_context(tc.tile_pool(name="sm", bufs=3))
            pt_pool = ph3.enter_context(tc.tile_pool(name="pt", bufs=4))
            wop = ph3.enter_context(tc.tile_pool(name="wop", bufs=2))

            psc_pool = ph3.enter_context(
                tc.tile_pool(name="psc", bufs=2, space="PSUM"))
            pav_pool = ph3.enter_context(
                tc.tile_pool(name="pav", bufs=1, space="PSUM"))
            pbc_pool = ph3.enter_context(
                tc.tile_pool(name="pbc", bufs=1, space="PSUM"))
            for s in range(NSLOT):
                nkt = NKT_PROG[s]
                mask_base = 0 if s == 0 else 8
                qs = slice(s * CW, (s + 1) * CW)
                avn = {}
                if True:
                    for pair in range(NPAIR):
                        av = pav_pool.tile([65, 1024], F32, tag="av", name="av")
                        for kt in range(nkt):
                            kws = slice(kt * P, (kt + 1) * P)
                            psc = psc_pool.tile([P, 1024], F32, tag="sc",
                                                name="psc")
                            nc.tensor.matmul(psc[:, 0:512], kT[pair][0:64, kws],
                                             qT[pair][0:64, qs],
                                             start=True, stop=True)
                            nc.tensor.matmul(psc[:, 512:1024],
                                             kT[pair][64:128, kws],
                                             qT[pair][64:128, qs],
                                             start=True, stop=True)
                            pt = pt_pool.tile([P, 1024], BF, tag="pt", name="pt")
                            nc.scalar.activation(pt[:], psc[:], AF.Exp,
                                                 bias=zero_col[:], scale=SCALE)
                            jm = kt - mask_base
                            if 0 <= jm < 8:
                                mt = masks[(s, jm)]
                                nc.vector.tensor_tensor(pt[:, 0:512], pt[:, 0:512],
                                                        mt[:], ALU.mult)
                                nc.vector.tensor_tensor(pt[:, 512:1024],
                                                        pt[:, 512:1024],
                                                        mt[:], ALU.mult)
                            st = (kt == 0)
                            sp = (kt == nkt - 1)
                            nc.tensor.matmul(av[0:65, 0:512],
                                             v65[:, kt, pair, 0, :],
                                             pt[:, 0:512],
                                             start=st, stop=sp,
                                             skip_group_check=True)
                            nc.tensor.matmul(av[0:65, 512:1024],
                                             v65[:, kt, pair, 1, :],
                                             pt[:, 512:1024],
                                             start=st, stop=sp,
                                             skip_group_check=True)
                        # normalize: evict, broadcast sums, reciprocal, scale
                        an = sm_pool.tile([65, 1024], BF, tag="an", name="an")
                        nc.vector.tensor_copy(an[:], av[0:65, :])
                        bc = pbc_pool.tile([64, 1024], F32, tag="bc", name="bc")
                        nc.tensor.matmul(bc[:, 0:512], onesP[64:65, 0:64],
                                         an[64:65, 0:512], start=True, stop=True)
                        nc.tensor.matmul(bc[:, 512:1024], onesP[64:65, 0:64],
                                         an[64:65, 512:1024], start=True, stop=True)
                        rec = sm_pool.tile([64, 1024], BF, tag="rec", name="rec")
                        lnd = sm_pool.tile([64, 1024], F32, tag="lnd", name="lnd")
                        nc.scalar.activation(lnd[:], bc[:], AF.Ln)
                        nc.scalar.activation(rec[:], lnd[:], AF.Exp, scale=-1.0)
                        anp = avn_pool.tile([P, CW], BF, tag=f"avn{pair}",
                                            name=f"avn{pair}")
                        nc.vector.tensor_tensor(anp[0:64, :], an[0:64, 0:512],
                                                rec[:, 0:512], ALU.mult)
                        tmo = sm_pool.tile([64, CW], BF, tag="tmo", name="tmo")
                        nc.vector.tensor_tensor(tmo[:], an[0:64, 512:1024],
                                                rec[:, 512:1024], ALU.mult)
                        nc.sync.dma_start(out=anp[64:128, :], in_=tmo[:])
                        avn[pair] = anp
                # Wo for this slot (bias bo folded into eviction); psum
                # shares the bc slot so both slots pipeline in 8 banks
                for m in range(NPAIR):
                    wom = wop.tile([P, KC, P], BF, tag="wom", name="wom")
                    nc.sync.dma_start(
                        out=wom[:],
                        in_=wo_d[m].rearrange("p (k m) -> p k m", k=KC))
                    ps = pbc_pool.tile([P, CW], F32, tag="bc", name="wops")
                    for k in range(NPAIR):
                        nc.tensor.matmul(ps[:], wom[:, k, :], avn[k][:],
                                         start=(k == 0), stop=(k == NPAIR - 1))
                    nc.vector.tensor_scalar(oT[m][:, s, :], ps[:],
                                            bot[:, m:m + 1], None, ALU.add)

        att_ctx.close()

        # =====================================================================
        # Phase 4: x2 = oT + xTq ; LN2 (not in place)
        # =====================================================================
        p_x2t = top.enter_context(tc.tile_pool(name="p_x2t", bufs=1))
        p_ln2T = top.enter_context(tc.tile_pool(name="p_ln2T", bufs=1))
        x2T = []
        ln2T_tiles = []
        for kc in range(KC):
            x2 = p_x2t.tile([P, NQ], BF, tag=f"x2t{kc}", name=f"x2t{kc}")
            nc.vector.tensor_tensor(
                x2[:], xTq[kc][:],
                oT[kc][:].rearrange("p s w -> p (s w)"), ALU.add)
            x2T.append(x2)
            lt = p_ln2T.tile([P, NQ], BF, tag=f"ln2T{kc}", name=f"ln2T{kc}")
            ln2T_tiles.append(lt)
        ln2T = layernorm_T(x2T, NQ, "g2", "be2", "ln2T", out_tiles=ln2T_tiles)

        # =====================================================================
        # Phase 5: FFN in two hidden-dim halves
        # =====================================================================
        with ExitStack() as ph5:
            ff1_pool = ph5.enter_context(tc.tile_pool(name="ff1", bufs=1))
            facc_pool = ph5.enter_context(tc.tile_pool(name="facc", bufs=1))
            w1_pool = ph5.enter_context(tc.tile_pool(name="w1s", bufs=2))
            w2_pool = ph5.enter_context(tc.tile_pool(name="w2s", bufs=2))
            fst_pool = ph5.enter_context(tc.tile_pool(name="fst", bufs=3))
            pF = ph5.enter_context(tc.tile_pool(name="pF", bufs=4, space="PSUM"))
            ffacc = [facc_pool.tile([P, NQ], BF, tag=f"facc{m}", name=f"ffacc{m}")
                     for m in range(KC)]
            for half in range(2):
                ff1 = []
                for m in range(16):
                    mm = half * 16 + m
                    w1m = w1_pool.tile([P, KC, P], BF, tag="w1m", name="w1m")
                    nc.sync.dma_start(
                        out=w1m[:],
                        in_=w1_d[mm].rearrange("p (k m) -> p k m", k=KC))
                    f = ff1_pool.tile([P, NQ], BF, tag=f"f{m}", name=f"f{m}")
                    for tch in range(2):
                        s = slice(tch * 512, (tch + 1) * 512)
                        ps = pF.tile([P, 512], F32, tag="proj", name="ps")
                        for kc in range(KC):
                            nc.tensor.matmul(ps[:], w1m[:, kc, :], ln2T[kc][:, s],
                                             start=(kc == 0), stop=(kc == KC - 1))
                        # relu(x + b1) eviction
                        nc.vector.tensor_scalar(f[:, s], ps[:], b1t[:, mm:mm + 1],
                                                0.0, ALU.add, ALU.max)
                    ff1.append(f)
                for mc in range(KC):
                    w2m = w2_pool.tile([P, 16, P], BF, tag="w2m", name="w2m")
                    nc.sync.dma_start(
                        out=w2m[:],
                        in_=w2_d[mc][:, half * 2048:(half + 1) * 2048]
                        .rearrange("p (k m) -> p k m", k=16))
                    for tch in range(2):
                        s = slice(tch * 512, (tch + 1) * 512)
                        ps = pF.tile([P, 512], F32, tag="proj", name="ps")
                        for kt in range(16):
                            nc.tensor.matmul(ps[:], w2m[:, kt, :], ff1[kt][:, s],
                                             start=(kt == 0), stop=(kt == 15))
                        if half == 0:
                            nc.scalar.activation(ffacc[mc][:, s], ps[:], AF.Copy)
                        else:
                            o = fst_pool.tile([P, 512], F32, tag="fo", name="fo")
                            nc.vector.tensor_scalar(o[:], ps[:], b2t[:, mc:mc + 1],
                                                    None, ALU.add)
                            nc.vector.tensor_tensor(o[:], o[:], ffacc[mc][:, s],
                                                    ALU.add)
                            nc.vector.tensor_tensor(o[:], o[:], x2T[mc][:, s],
                                                    ALU.add)
                            nc.sync.dma_start(out=out_d[mc * P:(mc + 1) * P, s],
                                              in_=o[:])

    nc.compile()
    return nc, names


_CACHE = {}


def _get_built():
    if "nc" not in _CACHE:
        _CACHE["nc"], _CACHE["names"] = _build()
    return _CACHE["nc"], _CACHE["names"]


def _host_inputs(x, Wq, Wk, Wv, Wo, bo, ln1_g, ln1_b, ln2_g, ln2_b, W1, b1, W2, b2):
    """Build the 8 per-core input maps (host work = sharding/layout only)."""
    from ml_dtypes import bfloat16
    f = np.float32

    def wtile(W, nmb, nkc):
        # [mb, p, kc*P_or_512] with [mb,p,kc*w+j] = W[kc*P+p, mb*wout+j]
        kin, cout = W.shape
        wout = cout // nmb
        return np.ascontiguousarray(
            W.reshape(nkc, P, nmb, wout).transpose(2, 1, 0, 3)
            .reshape(nmb, P, nkc * wout).astype(bfloat16))

    shared = {
        "wq": wtile(np.asarray(Wq, f), NPAIR, KC),
        "wk": wtile(np.asarray(Wk, f), NPAIR, KC),
        "wv": wtile(np.asarray(Wv, f), 2, KC),
        "wo": wtile(np.asarray(Wo, f), NPAIR, KC),
        "w1": wtile(np.asarray(W1, f), 32, KC),
        "w2": wtile(np.asarray(W2, f), NPAIR, 32),
        "bot": np.ascontiguousarray(np.asarray(bo, f).reshape(KC, P).T),
        "b1t": np.ascontiguousarray(np.asarray(b1, f).reshape(32, P).T),
        "b2t": np.ascontiguousarray(np.asarray(b2, f).reshape(KC, P).T),
        "g1c": np.ascontiguousarray(np.asarray(ln1_g, f).reshape(KC, P).T),
        "be1c": np.ascontiguousarray(np.asarray(ln1_b, f).reshape(KC, P).T),
        "g2c": np.ascontiguousarray(np.asarray(ln2_g, f).reshape(KC, P).T),
        "be2c": np.ascontiguousarray(np.asarray(ln2_b, f).reshape(KC, P).T),
        "onesC": np.ones((P, 1), bfloat16),
        "onesP": np.ones((P, P), bfloat16),
    }
    kl = np.arange(P)[:, None]
    ql = np.arange(CW)[None, :]
    in_maps = []
    for c in range(8):
        b, r = c // 2, c % 2
        gs = GSETS[r]
        xTb = np.ascontiguousarray(np.asarray(x[b], f).T.astype(bfloat16))
        qcols = np.concatenate([np.arange(CW * g, CW * (g + 1)) for g in gs])
        xTq = np.ascontiguousarray(xTb[:, qcols])
        m = np.empty((NSLOT, 8, P, CW), bfloat16)
        for s in range(NSLOT):
            q0 = CW * gs[s]
            base = 0 if s == 0 else 8
            for j in range(8):
                kt = base + j
                m[s, j] = ((P * kt + kl) <= (q0 + ql)).astype(bfloat16)
        im = dict(shared)
        im["xT"] = xTb
        im["xTq"] = xTq
        im["masks"] = m
        in_maps.append(im)
    return in_maps


def _unshard(outs):
    out = np.empty((4, T, C), np.float32)
    for c in range(8):
        b, r = c // 2, c % 2
        oT = outs[c]  # (C, NQ)
        for s, g in enumerate(GSETS[r]):
            out[b, CW * g:CW * (g + 1), :] = oT[:, CW * s:CW * (s + 1)].T
    return out


def kernel(**inputs):
    from concourse.bass_utils import run_bass_kernel_spmd
    from concourse.bass_interp import get_hw_module

    args = {k: np.asarray(v, np.float32) for k, v in inputs.items()}
    in_maps_named = _host_inputs(**args)

    nc, names = _get_built()
    in_maps = [{names[k]: v for k, v in im.items()} for im in in_maps_named]

    hw = get_hw_module(nc.m)
    old = nc.m
    nc.m = hw
    try:
        res = run_bass_kernel_spmd(nc, in_maps, core_ids=list(range(8)))
    finally:
        nc.m = old
    outs = [r[names["out"]] for r in res.results]
    return _unshard(outs)


if __name__ == "__main__":
    import reference
    inp = {k: np.asarray(v) for k, v in reference.setup_inputs().items()}
    got = kernel(**inp)
    exp = np.asarray(reference.reference(**inp))
    err = np.abs(got - exp).max() / np.abs(exp).max()
    print("Relative error:", err)
